# revision 16
# baseline (speedup 1.0000x reference)
"""Trainium2 Bass kernel: attention layer with KV cache, tensor-parallel over heads.

Sharding (8 NeuronCores): Megatron-style TP over the 32 heads -> 4 heads/core.
  - wq/wk/wv: column-parallel (each core owns a [512, 4096] output shard)
  - wo: row-parallel (each core owns wo[:, c*512:(c+1)*512]); cores emit
    partial o-proj outputs which the host sums (RowParallel unshard).
  - cache_k/cache_v: sharded along the head axis; history rows/positions are
    gathered host-side from batch_exec/start_pos (pure indexing).

Device layout trick: Q/K are projected directly in transposed [head_dim, tok]
layout (head dim = PSUM partitions), V in natural [tok, head_dim] layout, so
scores^T, PV, and the o-projection all consume each other's outputs as
matmul operands with zero on-device transposes.  Softmax runs without the
max-subtraction (scores are O(1), exp is safe in f32) so the kv-axis
(partition-axis) row-sum comes from a ones-vector matmul; normalization is a
rank-1 broadcast matmul of 1/r.  exp runs IN-PLACE in PSUM: the ACT engine's
fast port is PSUM on both sides (the ACT->SBUF write path measured ~8x
slower per element).

New-token compaction: router[:, :, 0] gates ~half the new cache positions
off for every query, so their K/V projections and score/PV columns are dead
work.  The host compacts each batch's unmasked new tokens (order preserved)
and the program only projects/attends over NKV = pad64(max_b count_b)
columns.  The program is built per-NKV (cached); causality in compact
coordinates is handled by the per-element gate plus the conservative
narrowing s0 = j0 (valid since orig_pos(j) >= j).
"""

import numpy as np
import ml_dtypes

import concourse.bass as bass
import concourse.bacc as bacc
import concourse.tile as tile
from concourse import mybir
from concourse.bass_utils import run_bass_kernel_spmd

BF16 = np.dtype(ml_dtypes.bfloat16)

# Problem shape (hardcoded per the task contract)
BSZ = 8
SEQ = 512
DIM = 4096
NH = 32
HD = 128
START = 512
KV = START + SEQ          # 1024
NC = 8                    # cores
HPC = NH // NC            # 4 heads per core
HF = HPC * HD             # 512 local features
P = 128
KC = DIM // P             # 32 contraction chunks
SC = SEQ // P             # 4 seq chunks (also history kv chunks)
ROPE_BASE = 10000.0

FP32 = mybir.dt.float32
BF16D = mybir.dt.bfloat16


def build_program(nkvs, newpos):
    # Every DRAM parameter is pre-packed host-side into the exact SBUF tile
    # layout (partition-major), so each DMA is a contiguous >=8KB-per-partition
    # stream: ~128 fat descriptors instead of thousands of 256B ones.
    # nkvs: per-batch padded compacted new-token counts (multiples of 16).
    # newpos[b][j] = original position of compacted token j (for exact causal
    # narrowing): new chunk starting at compact j0 only reaches queries
    # s >= newpos[b][j0].
    nkvm = max(nkvs)
    tcnm = (nkvm + P - 1) // P        # max new-token kv chunks
    nchm = SC + tcnm                  # max total kv chunks per head
    vchm = (nkvm + P - 1) // P        # max v chunks (tokens on partitions)

    nc = bacc.Bacc(None, target_bir_lowering=False)
    x_d = nc.declare_dram_parameter("xp", [BSZ, P, KC, SEQ], BF16D, isOutput=False)
    xkv_d = nc.declare_dram_parameter("xkvp", [BSZ, P, KC, nkvm], BF16D, isOutput=False)
    wqk_d = nc.declare_dram_parameter("wqk", [2 * HPC, P, KC, P], BF16D, isOutput=False)
    wv_d = nc.declare_dram_parameter("wvp", [P, KC, HF], BF16D, isOutput=False)
    woT_d = nc.declare_dram_parameter("wop", [P, HPC, DIM], BF16D, isOutput=False)
    ropeq_d = nc.declare_dram_parameter("ropeq", [BSZ, P, 2, SEQ], FP32, isOutput=False)
    ropek_d = nc.declare_dram_parameter("ropek", [BSZ, P, 2, nkvm], FP32, isOutput=False)
    kTh_d = nc.declare_dram_parameter("kThp", [BSZ, P, HPC, START], BF16D, isOutput=False)
    vh_d = nc.declare_dram_parameter("vhp", [BSZ, P, HPC * SC, HD], BF16D, isOutput=False)
    gateT_d = nc.declare_dram_parameter("gatep", [BSZ, P, nchm, SEQ], BF16D, isOutput=False)
    out_d = nc.declare_dram_parameter("out", [BSZ * SEQ, DIM], BF16D, isOutput=True)

    from contextlib import ExitStack

    with ExitStack() as ctx:
        tc = ctx.enter_context(tile.TileContext(nc))
        cpool = ctx.enter_context(tc.tile_pool(name="const", bufs=1))
        wupool = ctx.enter_context(tc.tile_pool(name="wu", bufs=4))
        xpool = ctx.enter_context(tc.tile_pool(name="xb", bufs=1))
        rpool = ctx.enter_context(tc.tile_pool(name="rope", bufs=1))
        qkvpool = ctx.enter_context(tc.tile_pool(name="qkv", bufs=2))
        hpool = ctx.enter_context(tc.tile_pool(name="hist", bufs=1))
        wkpool = ctx.enter_context(tc.tile_pool(name="work", bufs=2))
        epool = ctx.enter_context(tc.tile_pool(name="ee", bufs=2))
        apool = ctx.enter_context(tc.tile_pool(name="at", bufs=2))
        smpool = ctx.enter_context(tc.tile_pool(name="small", bufs=1))
        gpool = ctx.enter_context(tc.tile_pool(name="gate", bufs=1))
        popool = ctx.enter_context(tc.tile_pool(name="po", bufs=2))
        pA = ctx.enter_context(tc.tile_pool(name="pA", bufs=2, space="PSUM"))
        pS = ctx.enter_context(tc.tile_pool(name="pS", bufs=2, space="PSUM"))
        pR = ctx.enter_context(tc.tile_pool(name="pR", bufs=1, space="PSUM"))
        pOT = ctx.enter_context(tc.tile_pool(name="pOT", bufs=1, space="PSUM"))
        pP = ctx.enter_context(tc.tile_pool(name="pP", bufs=2, space="PSUM"))
        if True:
            # ---- constants (weights emitted after b0's hot DMAs, see below) ----
            ones_bf = cpool.tile([P, 1], BF16D)
            nc.gpsimd.memset(ones_bf[:], 1.0)
            wv_s = cpool.tile([P, KC, HF], BF16D)
            woT_s = cpool.tile([P, HPC, DIM], BF16D)

            def oproj_unit(aT_prev, ts_prev, k):
                # one (seq-chunk, out-chunk-pair) group of the previous
                # batch's o-projection: dense PE filler between the
                # exp/gate-gated attention beats.
                sc4, og = divmod(k, 4)
                pout = popool.tile([P, 2, HF], BF16D, tag="pout")
                for oi in range(2):
                    oc = og * 2 + oi
                    pp = pP.tile([P, HF], FP32, tag="pp")
                    for h in range(HPC):
                        nc.tensor.matmul(
                            pp[:],
                            aT_prev[:, h, sc4 * P:(sc4 + 1) * P],
                            woT_s[:, h, oc * HF:(oc + 1) * HF],
                            start=(h == 0), stop=(h == HPC - 1),
                        )
                    nc.vector.tensor_copy(pout[:, oi, :], pp[:])
                nc.gpsimd.dma_start(
                    out_d[ts_prev + sc4 * P: ts_prev + (sc4 + 1) * P,
                          og * 2 * HF:(og * 2 + 2) * HF],
                    pout[:],
                )

            prev_o = None  # (aT, ts) of the batch whose o-proj is pending
            for b in range(BSZ):
                ts = b * SEQ
                nkv = nkvs[b]
                tcn = (nkv + P - 1) // P
                nch = SC + tcn
                vch = (nkv + P - 1) // P
                # hoist the first weight unit ahead of the 4MB x stream so
                # the next phase-A matmul group starts ~1.5MB into the DMA
                wuf0 = wupool.tile([P, 16, P], BF16D, tag="wu")
                wuf1 = wupool.tile([P, 16, P], BF16D, tag="wu")
                wu_first = [wuf0, wuf1]
                for wc in range(4):
                    nc.sync.dma_start(wu_first[wc // 2][:, (wc % 2) * 8:(wc % 2 + 1) * 8, :],
                                      wqk_d[0, :, wc * 8:(wc + 1) * 8, :])
                xb = xpool.tile([P, KC, SEQ], BF16D)
                for xc in range(4):  # split so the first matmuls start early
                    nc.sync.dma_start(
                        xb[:, xc * 8:(xc + 1) * 8, :],
                        x_d[b, :, xc * 8:(xc + 1) * 8, :],
                    )
                xkvb = xpool.tile([P, KC, nkvm], BF16D, tag="xkv")
                # small hot tensors ride the scalar engine's DGE queues so
                # they are not stuck behind the multi-MB sync-queue streams
                ropeq_b = rpool.tile([P, 2, SEQ], FP32, tag="rq")
                nc.scalar.dma_start(ropeq_b[:], ropeq_d[b])
                ropek_b = rpool.tile([P, 2, nkvm], FP32, tag="rk")
                nc.scalar.dma_start(ropek_b[:, :, :nkv], ropek_d[b, :, :, :nkv])
                kThb = hpool.tile([P, HPC, START], BF16D)
                nc.scalar.dma_start(kThb[:], kTh_d[b])
                vhb = hpool.tile([P, HPC * SC, HD], BF16D)
                nc.scalar.dma_start(vhb[:], vh_d[b])
                gtb = gpool.tile([P, nchm, SEQ], BF16D)
                nc.scalar.dma_start(gtb[:, :nch, :], gateT_d[b, :, :nch, :])
                if b == 0:
                    nc.scalar.dma_start(woT_s[:, 0:2, :], woT_d[:, 0:2, :])
                    nc.scalar.dma_start(woT_s[:, 2:4, :], woT_d[:, 2:4, :])

                # ---- phase A: QKV projections (+RoPE for q/k) ----
                qT_b = qkvpool.tile([P, HPC, SEQ], BF16D, tag="qT")
                kT_b = qkvpool.tile([P, HPC, nkvm], BF16D, tag="kT")
                v_b = qkvpool.tile([P, vchm, HF], BF16D, tag="v", bufs=1)

                for proj in range(2):  # 0=q (full tokens), 1=k (compacted)
                    if proj == 1:
                        # xkv is only needed by the k/v units; emitting its
                        # DMA here keeps the q weight-units at the head of
                        # the sync queue (NOTE: the scalar DGE queue is
                        # bandwidth-limited -- bulk streams belong on sync)
                        for xc in range(2):
                            nc.sync.dma_start(
                                xkvb[:, xc * 16:(xc + 1) * 16, :nkv],
                                xkv_d[b, :, xc * 16:(xc + 1) * 16, :nkv],
                            )
                    if b == 0 and proj == 1:
                        # consts queue on the sync FIFO only after b0's
                        # q-units, so the critical startup stream drains first
                        nc.sync.dma_start(wv_s[:, 0:16, :], wv_d[:, 0:16, :])
                        nc.sync.dma_start(wv_s[:, 16:32, :], wv_d[:, 16:32, :])
                    if proj == 0:
                        dst, src, rope_t, ncols = qT_b, xb, ropeq_b, SEQ
                    else:
                        dst, src, rope_t, ncols = kT_b, xkvb, ropek_b, nkv
                    for h in range(HPC):
                        if proj == 0 and h == 0:
                            wu = wu_first
                        else:
                            wu0 = wupool.tile([P, 16, P], BF16D, tag="wu")
                            wu1 = wupool.tile([P, 16, P], BF16D, tag="wu")
                            wu = [wu0, wu1]
                            for half in range(2):
                                u = proj * HPC + h
                                nc.sync.dma_start(wu[half][:, 0:8, :], wqk_d[u, :, half * 16:half * 16 + 8, :])
                                nc.sync.dma_start(wu[half][:, 8:16, :], wqk_d[u, :, half * 16 + 8:half * 16 + 16, :])
                        ps = pA.tile([P, SEQ], FP32, tag="pa")
                        for kc in range(KC):
                            nc.tensor.matmul(
                                ps[:, :ncols], wu[kc // 16][:, kc % 16, :], src[:, kc, :ncols],
                                start=(kc == 0), stop=(kc == KC - 1),
                            )
                        # RoPE: dst = ps*cos + shift64(ps)*sin_signed
                        t1 = wkpool.tile([P, SEQ], FP32, tag="t1")
                        nc.vector.tensor_mul(t1[:, :ncols], ps[:, :ncols], rope_t[:, 0, :ncols])
                        t2 = wkpool.tile([P, SEQ], FP32, tag="t2")
                        H2 = HD // 2
                        nc.vector.tensor_mul(t2[0:H2, :ncols], ps[H2:P, :ncols], rope_t[0:H2, 1, :ncols])
                        nc.vector.tensor_mul(t2[H2:P, :ncols], ps[0:H2, :ncols], rope_t[H2:P, 1, :ncols])
                        nc.vector.tensor_add(dst[:, h, :ncols], t1[:, :ncols], t2[:, :ncols])

                for vc in range(vch):  # v, natural layout, compacted tokens
                    pc = min(P, nkv - vc * P)
                    ps = pA.tile([P, SEQ], FP32, tag="pa")
                    for kc in range(KC):
                        nc.tensor.matmul(
                            ps[0:pc, :HF], xkvb[:, kc, vc * P:vc * P + pc], wv_s[:, kc, :],
                            start=(kc == 0), stop=(kc == KC - 1),
                        )
                    nc.vector.tensor_copy(v_b[0:pc, vc, :], ps[0:pc, :HF])

                # ---- phase B: attention, software-pipelined per head ----
                # Chunk list: 4 history chunks (s0=0) then compacted new
                # chunks; new chunk at compact offset j0 only reaches queries
                # s >= j0 (orig position >= compact index), so narrow ops.
                # The PE queue is strict FIFO, so the emission order below IS
                # the execution order: rowsum/PV for chunk ci-2 are emitted
                # between score matmuls so the PE never heads-of-line blocks
                # on the exp->gate chain, and o-proj groups of the PREVIOUS
                # batch are sprinkled in as dense filler.
                aT = apool.tile([P, HPC, SEQ], BF16D)
                ounits = list(range(16)) if prev_o is not None else []
                for h in range(HPC):
                    chunks = []
                    for t in range(SC):
                        chunks.append((kThb[:, h, t * P:(t + 1) * P],
                                       vhb[:, h * SC + t, :], P, 0))
                    for tcn_i in range(tcn):
                        j0 = tcn_i * P
                        pc = min(P, nkv - j0)
                        s0 = int(newpos[b][j0]) if j0 < len(newpos[b]) else SEQ - 1
                        chunks.append((kT_b[:, h, j0:j0 + pc],
                                       v_b[0:pc, tcn_i, h * P:(h + 1) * P], pc, s0))
                    ee = epool.tile([P, nchm, SEQ], BF16D)
                    pr = pR.tile([1, SEQ], FP32, tag="pr")
                    po = pOT.tile([P, SEQ], FP32, tag="po")

                    def rs_pv(ci):
                        _, vlhs, pc, s0 = chunks[ci]
                        nc.tensor.matmul(
                            pr[:, s0:], ones_bf[0:pc, :], ee[0:pc, ci, s0:],
                            start=(ci == 0), stop=(ci == nch - 1),
                            skip_group_check=True,
                        )
                        nc.tensor.matmul(
                            po[:, s0:], vlhs, ee[0:pc, ci, s0:],
                            start=(ci == 0), stop=(ci == nch - 1),
                            skip_group_check=True,
                        )

                    for ci, (klhs, vlhs, pc, s0) in enumerate(chunks):
                        pscr = pS.tile([P, SEQ], FP32, tag="ps")
                        nc.tensor.matmul(pscr[0:pc, s0:], klhs, qT_b[:, h, s0:], start=True, stop=True)
                        # exp in-place in PSUM: ScalarE's fast port is PSUM on
                        # both sides; ACT->SBUF measured ~8x slower.
                        nc.scalar.activation(pscr[0:pc, s0:], pscr[0:pc, s0:], mybir.ActivationFunctionType.Exp)
                        nc.vector.tensor_mul(ee[0:pc, ci, s0:], pscr[0:pc, s0:], gtb[0:pc, ci, s0:])
                        if ci >= 2:
                            rs_pv(ci - 2)
                            if ci % 2 == 0 and ounits:
                                oproj_unit(prev_o[0], prev_o[1], ounits.pop(0))
                    rs_pv(nch - 2)
                    if ounits:
                        oproj_unit(prev_o[0], prev_o[1], ounits.pop(0))
                    rs_pv(nch - 1)
                    rinv = smpool.tile([1, SEQ], FP32, tag="rinv")
                    nc.vector.reciprocal_approx_fast(rinv[:], pr[:])
                    rb_s = smpool.tile([P, SEQ], FP32, tag="rbs")
                    nc.gpsimd.partition_broadcast(rb_s[:], rinv[:])
                    nc.vector.tensor_mul(aT[:, h, :], po[:], rb_s[:])
                # drain any o-proj groups this batch's slots didn't absorb
                while ounits:
                    oproj_unit(prev_o[0], prev_o[1], ounits.pop(0))
                prev_o = (aT, ts)

            # final batch's o-projection (no next attention phase to hide in)
            for k in range(16):
                oproj_unit(prev_o[0], prev_o[1], k)
    nc.finalize()
    return nc


_CACHE = {}


def _get_program(nkvs, newpos):
    key = (nkvs, tuple(tuple(int(v) for v in p) for p in newpos))
    if key not in _CACHE:
        _CACHE[key] = build_program(nkvs, newpos)
    return _CACHE[key]


def _prep_inputs(inputs):
    x = np.asarray(inputs["x"], np.float32)
    router = np.asarray(inputs["router"], np.float32)
    cache_k = np.asarray(inputs["cache_k"], np.float32)
    cache_v = np.asarray(inputs["cache_v"], np.float32)
    cache_mask = np.asarray(inputs["cache_mask"])
    mask = np.asarray(inputs["mask"], np.float32)
    wq = np.asarray(inputs["wq"], np.float32)
    wk = np.asarray(inputs["wk"], np.float32)
    wv = np.asarray(inputs["wv"], np.float32)
    wo = np.asarray(inputs["wo"], np.float32)
    position_ids = np.asarray(inputs["position_ids"], np.int64)
    batch_exec = np.asarray(inputs["batch_exec"], np.int64)
    start_pos = int(inputs["start_pos"])
    assert start_pos == START and x.shape == (BSZ, SEQ, DIM)

    # compacted new-token index lists (order-preserving)
    pen_new = router[:, :, 0] != 0.0                                  # [8, 512]
    idx = [np.nonzero(pen_new[b])[0] for b in range(BSZ)]
    nkvs = tuple(max(16, ((len(i) + 15) // 16) * 16) for i in idx)
    nkv = max(nkvs)
    tcn = (nkv + P - 1) // P
    nch = SC + tcn

    # x packed per batch into the SBUF tile layout [b, p, kc, tok]
    xT = x.reshape(BSZ, SEQ, KC, P)               # tok-major view of features
    xp = np.ascontiguousarray(xT.transpose(0, 3, 2, 1)).astype(BF16)  # [8,128,32,512]
    xsel = np.zeros((BSZ, nkv, KC, P), np.float32)
    for b in range(BSZ):
        xsel[b, :len(idx[b])] = xT[b, idx[b]]
    xkvp = np.ascontiguousarray(xsel.transpose(0, 3, 2, 1)).astype(BF16)

    # RoPE tables gathered at position_ids, packed [b, p(hd), table, tok]
    inv_freq = 1.0 / (ROPE_BASE ** (np.arange(0, HD, 2, dtype=np.float32) / HD))
    t = np.arange(KV, dtype=np.float32)
    emb = np.concatenate([t[:, None] * inv_freq, t[:, None] * inv_freq], axis=-1)
    cos_t = np.cos(emb).astype(np.float32)[position_ids]   # [8, 512, 128]
    sin_t = np.sin(emb).astype(np.float32)[position_ids]
    sign = np.where(np.arange(HD) < HD // 2, -1.0, 1.0).astype(np.float32)
    scale = np.float32(1.0 / np.sqrt(HD))
    ropeq = np.stack([cos_t * scale, (sin_t * sign) * scale], axis=1)  # [8,2,512,128]
    ropeqp = np.ascontiguousarray(ropeq.transpose(0, 3, 1, 2)).astype(np.float32)
    ropek = np.zeros((BSZ, 2, nkv, HD), np.float32)
    for b in range(BSZ):
        nb = len(idx[b])
        ropek[b, 0, :nb] = cos_t[b, idx[b]]
        ropek[b, 1, :nb] = sin_t[b, idx[b]] * sign
    ropekp = np.ascontiguousarray(ropek.transpose(0, 3, 1, 2)).astype(np.float32)

    # history cache slices (host-side gather = sharding)
    k_hist = cache_k[batch_exec, :, :START, :]   # [8, 32, 512, 128]
    v_hist = cache_v[batch_exec, :, :START, :]

    # multiplicative 0/1 gate: causal AND cache-usable, with the new-token
    # half compacted to idx[b]; packed [b, p, chunk, s]
    pen_hist = cache_mask[batch_exec, :START].astype(bool)            # [8, 512]
    causal_ok = (mask[0, 0] > -0.5)                                   # [512 s, 1024 t]
    gate_hist = causal_ok.T[None, :START, :] & pen_hist[:, :, None]   # [8, 512, 512]
    gate_new = np.zeros((BSZ, nkv, SEQ), bool)
    for b in range(BSZ):
        nb = len(idx[b])
        gate_new[b, :nb] = causal_ok.T[START + idx[b], :]
    gate = np.concatenate([gate_hist, gate_new], axis=1)              # [8, 512+nkv, 512]
    pad = nch * P - gate.shape[1]
    if pad:
        gate = np.concatenate([gate, np.zeros((BSZ, pad, SEQ), bool)], axis=1)
    gatep = np.ascontiguousarray(
        gate.reshape(BSZ, nch, P, SEQ).transpose(0, 2, 1, 3)
        .astype(np.float32)).astype(BF16)                             # [8,128,nch,512]

    in_maps = []
    for c in range(NC):
        hs, he = c * HPC, (c + 1) * HPC
        fs, fe = c * HF, (c + 1) * HF
        # q/k units [2*HPC, p, kc, 128]: unit (proj, h) = W[fs+h*128 : ...].T
        wqkT = np.stack([w[fs:fe].T for w in (wq, wk)])   # [2, 4096, 512]
        wqk = (wqkT.reshape(2, KC, P, HPC, HD).transpose(0, 3, 2, 1, 4)
               .reshape(2 * HPC, P, KC, HD))
        wvT = wv[fs:fe].T                                  # [4096, 512]
        wvp = wvT.reshape(KC, P, HF).transpose(1, 0, 2)    # [128, 32, 512]
        woTc = wo[:, fs:fe].T                              # [512, 4096]
        wop = woTc.reshape(HPC, P, DIM).transpose(1, 0, 2) # [128, 4, 4096]
        kThp = k_hist[:, hs:he].transpose(0, 3, 1, 2)      # [8, 128hd, 4h, 512]
        vhp = (v_hist[:, hs:he].reshape(BSZ, HPC, SC, P, HD)
               .transpose(0, 3, 1, 2, 4).reshape(BSZ, P, HPC * SC, HD))
        in_maps.append({
            "xp": xp,
            "xkvp": xkvp,
            "wqk": np.ascontiguousarray(wqk).astype(BF16),
            "wvp": np.ascontiguousarray(wvp).astype(BF16),
            "wop": np.ascontiguousarray(wop).astype(BF16),
            "ropeq": ropeqp,
            "ropek": ropekp,
            "kThp": np.ascontiguousarray(kThp).astype(BF16),
            "vhp": np.ascontiguousarray(vhp).astype(BF16),
            "gatep": gatep,
        })
    return in_maps, nkvs, idx


def _install_profile_hook():
    """The agent image's antenv lacks axon_hooks; shim it so trace=True works."""
    import sys, types
    if "antenv.axon_hooks" in sys.modules:
        return
    try:
        from trn_agent_boot.trn_boot import _ntff_profile_via_ctypes
    except ImportError:
        return
    mod = types.ModuleType("antenv.axon_hooks")
    mod._hook = _ntff_profile_via_ctypes("/opt/axon/libaxon_pjrt.so")

    def set_axon_ntff_profile_hook(h):
        mod._hook = h

    def get_axon_ntff_profile_hook():
        return mod._hook

    mod.set_axon_ntff_profile_hook = set_axon_ntff_profile_hook
    mod.get_axon_ntff_profile_hook = get_axon_ntff_profile_hook
    sys.modules["antenv.axon_hooks"] = mod
    import antenv
    antenv.axon_hooks = mod


def _run(inputs, trace=False):
    if trace:
        _install_profile_hook()
    in_maps, nkvs, newpos = _prep_inputs(inputs)
    nc = _get_program(nkvs, newpos)
    res = run_bass_kernel_spmd(nc, in_maps, core_ids=list(range(NC)), trace=trace)
    acc = np.zeros((BSZ * SEQ, DIM), np.float32)
    for c in range(NC):
        acc += res.results[c]["out"].astype(np.float32)
    return acc.reshape(BSZ, SEQ, DIM), res


def kernel(**inputs):
    out, _ = _run(inputs, trace=False)
    return out


# revision 18
# speedup vs baseline: 1.1581x; 1.1581x over previous
"""Trainium2 Bass kernel: attention layer with KV cache, tensor-parallel over heads.

Sharding (8 NeuronCores): Megatron-style TP over the 32 heads -> 4 heads/core.
  - wq/wk/wv: column-parallel (each core owns a [512, 4096] output shard)
  - wo: row-parallel (each core owns wo[:, c*512:(c+1)*512]); cores emit
    partial o-proj outputs which the host sums (RowParallel unshard).
  - cache_k/cache_v: sharded along the head axis; history rows/positions are
    gathered host-side from batch_exec/start_pos (pure indexing).

Device layout trick: Q/K are projected directly in transposed [head_dim, tok]
layout (head dim = PSUM partitions), V in natural [tok, head_dim] layout, so
scores^T, PV, and the o-projection all consume each other's outputs as
matmul operands with zero on-device transposes.  Softmax runs without the
max-subtraction (scores are O(1), exp is safe in f32) so the kv-axis
(partition-axis) row-sum comes from a ones-vector matmul; normalization is a
rank-1 broadcast matmul of 1/r.  exp runs IN-PLACE in PSUM: the ACT engine's
fast port is PSUM on both sides (the ACT->SBUF write path measured ~8x
slower per element).

New-token compaction: router[:, :, 0] gates ~half the new cache positions
off for every query, so their K/V projections and score/PV columns are dead
work.  The host compacts each batch's unmasked new tokens (order preserved)
and the program projects/attends over pad16(count_b) columns per batch,
with exact causal narrowing from the compacted token positions (chunk at
compact offset j0 only reaches queries s >= orig_pos(j0)).  Batches with
>384 usable tokens fall back to uncompacted identity order (SBUF would not
fit the staging buffer, and compaction stops paying).  The program is built
per-(counts, positions) and cached, so repeated calls with the same router
compile once.

Scheduling: the PE queue is strict FIFO, so emission order is execution
order.  Attention is software-pipelined per head (rowsum/PV matmuls lag two
chunks behind the score matmuls), and the PREVIOUS batch's o-projection
groups are interleaved into the attention beats as dense PE filler, so the
PE never head-of-line blocks on the exp->gate chain.
"""

import numpy as np
import ml_dtypes

import concourse.bass as bass
import concourse.bacc as bacc
import concourse.tile as tile
from concourse import mybir
from concourse.bass_utils import run_bass_kernel_spmd

BF16 = np.dtype(ml_dtypes.bfloat16)

# Problem shape (hardcoded per the task contract)
BSZ = 8
SEQ = 512
DIM = 4096
NH = 32
HD = 128
START = 512
KV = START + SEQ          # 1024
NC = 8                    # cores
HPC = NH // NC            # 4 heads per core
HF = HPC * HD             # 512 local features
P = 128
KC = DIM // P             # 32 contraction chunks
SC = SEQ // P             # 4 seq chunks (also history kv chunks)
ROPE_BASE = 10000.0

FP32 = mybir.dt.float32
BF16D = mybir.dt.bfloat16


def build_program(nkvs, newpos, wu_bufs=4):
    # Every DRAM parameter is pre-packed host-side into the exact SBUF tile
    # layout (partition-major), so each DMA is a contiguous >=8KB-per-partition
    # stream: ~128 fat descriptors instead of thousands of 256B ones.
    # nkvs: per-batch padded compacted new-token counts (multiples of 16).
    # newpos[b][j] = original position of compacted token j (for exact causal
    # narrowing): new chunk starting at compact j0 only reaches queries
    # s >= newpos[b][j0].
    nkvm = max(nkvs)
    tcnm = (nkvm + P - 1) // P        # max new-token kv chunks
    nchm = SC + tcnm                  # max total kv chunks per head
    vchm = (nkvm + P - 1) // P        # max v chunks (tokens on partitions)
    # batches with nkv == SEQ run uncompacted straight from the resident x
    # tile; the compacted-x staging buffer only needs the compacted batches
    nkvc = max([n for n in nkvs if n < SEQ], default=16)

    nc = bacc.Bacc(None, target_bir_lowering=False)
    x_d = nc.declare_dram_parameter("xp", [BSZ, P, KC, SEQ], BF16D, isOutput=False)
    xkv_d = nc.declare_dram_parameter("xkvp", [BSZ, P, KC, nkvc], BF16D, isOutput=False)
    wqk_d = nc.declare_dram_parameter("wqk", [2 * HPC, P, KC, P], BF16D, isOutput=False)
    wv_d = nc.declare_dram_parameter("wvp", [P, KC, HF], BF16D, isOutput=False)
    woT_d = nc.declare_dram_parameter("wop", [P, HPC, DIM], BF16D, isOutput=False)
    ropeq_d = nc.declare_dram_parameter("ropeq", [BSZ, P, 2, SEQ], FP32, isOutput=False)
    ropek_d = nc.declare_dram_parameter("ropek", [BSZ, P, 2, nkvm], FP32, isOutput=False)
    kTh_d = nc.declare_dram_parameter("kThp", [BSZ, P, HPC, START], BF16D, isOutput=False)
    vh_d = nc.declare_dram_parameter("vhp", [BSZ, P, HPC * SC, HD], BF16D, isOutput=False)
    gateT_d = nc.declare_dram_parameter("gatep", [BSZ, P, nchm, SEQ], BF16D, isOutput=False)
    out_d = nc.declare_dram_parameter("out", [BSZ * SEQ, DIM], BF16D, isOutput=True)

    from contextlib import ExitStack

    with ExitStack() as ctx:
        tc = ctx.enter_context(tile.TileContext(nc))
        cpool = ctx.enter_context(tc.tile_pool(name="const", bufs=1))
        wupool = ctx.enter_context(tc.tile_pool(name="wu", bufs=wu_bufs))
        xpool = ctx.enter_context(tc.tile_pool(name="xb", bufs=1))
        rpool = ctx.enter_context(tc.tile_pool(name="rope", bufs=1))
        qkvpool = ctx.enter_context(tc.tile_pool(name="qkv", bufs=2))
        hpool = ctx.enter_context(tc.tile_pool(name="hist", bufs=1))
        wkpool = ctx.enter_context(tc.tile_pool(name="work", bufs=2))
        epool = ctx.enter_context(tc.tile_pool(name="ee", bufs=2))
        apool = ctx.enter_context(tc.tile_pool(name="at", bufs=2))
        smpool = ctx.enter_context(tc.tile_pool(name="small", bufs=1))
        gpool = ctx.enter_context(tc.tile_pool(name="gate", bufs=1))
        popool = ctx.enter_context(tc.tile_pool(name="po", bufs=2))
        pA = ctx.enter_context(tc.tile_pool(name="pA", bufs=2, space="PSUM"))
        pS = ctx.enter_context(tc.tile_pool(name="pS", bufs=2, space="PSUM"))
        pR = ctx.enter_context(tc.tile_pool(name="pR", bufs=1, space="PSUM"))
        pOT = ctx.enter_context(tc.tile_pool(name="pOT", bufs=1, space="PSUM"))
        pP = ctx.enter_context(tc.tile_pool(name="pP", bufs=2, space="PSUM"))
        if True:
            # ---- constants (weights emitted after b0's hot DMAs, see below) ----
            ones_bf = cpool.tile([P, 1], BF16D)
            nc.gpsimd.memset(ones_bf[:], 1.0)
            wv_s = cpool.tile([P, KC, HF], BF16D)
            woT_s = cpool.tile([P, HPC, DIM], BF16D)

            def oproj_unit(aT_prev, ts_prev, k):
                # one (seq-chunk, out-chunk-pair) group of the previous
                # batch's o-projection: dense PE filler between the
                # exp/gate-gated attention beats.
                sc4, og = divmod(k, 4)
                pout = popool.tile([P, 2, HF], BF16D, tag="pout")
                for oi in range(2):
                    oc = og * 2 + oi
                    pp = pP.tile([P, HF], FP32, tag="pp")
                    for h in range(HPC):
                        nc.tensor.matmul(
                            pp[:],
                            aT_prev[:, h, sc4 * P:(sc4 + 1) * P],
                            woT_s[:, h, oc * HF:(oc + 1) * HF],
                            start=(h == 0), stop=(h == HPC - 1),
                        )
                    nc.vector.tensor_copy(pout[:, oi, :], pp[:])
                nc.gpsimd.dma_start(
                    out_d[ts_prev + sc4 * P: ts_prev + (sc4 + 1) * P,
                          og * 2 * HF:(og * 2 + 2) * HF],
                    pout[:],
                )

            prev_o = None  # (aT, ts) of the batch whose o-proj is pending
            for b in range(BSZ):
                ts = b * SEQ
                nkv = nkvs[b]
                tcn = (nkv + P - 1) // P
                nch = SC + tcn
                vch = (nkv + P - 1) // P
                # hoist the first weight unit ahead of the 4MB x stream so
                # the next phase-A matmul group starts ~1.5MB into the DMA
                wuf0 = wupool.tile([P, 16, P], BF16D, tag="wu")
                wuf1 = wupool.tile([P, 16, P], BF16D, tag="wu")
                wu_first = [wuf0, wuf1]
                for wc in range(4):
                    nc.sync.dma_start(wu_first[wc // 2][:, (wc % 2) * 8:(wc % 2 + 1) * 8, :],
                                      wqk_d[0, :, wc * 8:(wc + 1) * 8, :])
                xb = xpool.tile([P, KC, SEQ], BF16D)
                for xc in range(4):  # split so the first matmuls start early
                    nc.sync.dma_start(
                        xb[:, xc * 8:(xc + 1) * 8, :],
                        x_d[b, :, xc * 8:(xc + 1) * 8, :],
                    )
                xkvb = xpool.tile([P, KC, nkvc], BF16D, tag="xkv")
                # small hot tensors ride the scalar engine's DGE queues so
                # they are not stuck behind the multi-MB sync-queue streams
                ropeq_b = rpool.tile([P, 2, SEQ], FP32, tag="rq")
                nc.scalar.dma_start(ropeq_b[:], ropeq_d[b])
                ropek_b = rpool.tile([P, 2, nkvm], FP32, tag="rk")
                nc.scalar.dma_start(ropek_b[:, :, :nkv], ropek_d[b, :, :, :nkv])
                kThb = hpool.tile([P, HPC, START], BF16D)
                nc.scalar.dma_start(kThb[:], kTh_d[b])
                vhb = hpool.tile([P, HPC * SC, HD], BF16D)
                nc.scalar.dma_start(vhb[:], vh_d[b])
                gtb = gpool.tile([P, nchm, SEQ], BF16D)
                nc.scalar.dma_start(gtb[:, :nch, :], gateT_d[b, :, :nch, :])
                if b == 0:
                    nc.scalar.dma_start(woT_s[:, 0:2, :], woT_d[:, 0:2, :])
                    nc.scalar.dma_start(woT_s[:, 2:4, :], woT_d[:, 2:4, :])

                # ---- phase A: QKV projections (+RoPE for q/k) ----
                qT_b = qkvpool.tile([P, HPC, SEQ], BF16D, tag="qT")
                kT_b = qkvpool.tile([P, HPC, nkvm], BF16D, tag="kT")
                v_b = qkvpool.tile([P, vchm, HF], BF16D, tag="v", bufs=1)

                for proj in range(2):  # 0=q (full tokens), 1=k (compacted)
                    if proj == 1 and nkv < SEQ:
                        # xkv is only needed by the k/v units; emitting its
                        # DMA here keeps the q weight-units at the head of
                        # the sync queue (NOTE: the scalar DGE queue is
                        # bandwidth-limited -- bulk streams belong on sync)
                        for xc in range(2):
                            nc.sync.dma_start(
                                xkvb[:, xc * 16:(xc + 1) * 16, :nkv],
                                xkv_d[b, :, xc * 16:(xc + 1) * 16, :nkv],
                            )
                    if b == 0 and proj == 1:
                        # consts queue on the sync FIFO only after b0's
                        # q-units, so the critical startup stream drains first
                        nc.sync.dma_start(wv_s[:, 0:16, :], wv_d[:, 0:16, :])
                        nc.sync.dma_start(wv_s[:, 16:32, :], wv_d[:, 16:32, :])
                    if proj == 0:
                        dst, src, rope_t, ncols = qT_b, xb, ropeq_b, SEQ
                    else:
                        src = xb if nkv == SEQ else xkvb
                        dst, rope_t, ncols = kT_b, ropek_b, nkv
                    for h in range(HPC):
                        if proj == 0 and h == 0:
                            wu = wu_first
                        else:
                            wu0 = wupool.tile([P, 16, P], BF16D, tag="wu")
                            wu1 = wupool.tile([P, 16, P], BF16D, tag="wu")
                            wu = [wu0, wu1]
                            for half in range(2):
                                u = proj * HPC + h
                                nc.sync.dma_start(wu[half][:, 0:8, :], wqk_d[u, :, half * 16:half * 16 + 8, :])
                                nc.sync.dma_start(wu[half][:, 8:16, :], wqk_d[u, :, half * 16 + 8:half * 16 + 16, :])
                        ps = pA.tile([P, SEQ], FP32, tag="pa")
                        for kc in range(KC):
                            nc.tensor.matmul(
                                ps[:, :ncols], wu[kc // 16][:, kc % 16, :], src[:, kc, :ncols],
                                start=(kc == 0), stop=(kc == KC - 1),
                            )
                        # RoPE: dst = ps*cos + shift64(ps)*sin_signed
                        t1 = wkpool.tile([P, SEQ], FP32, tag="t1")
                        nc.vector.tensor_mul(t1[:, :ncols], ps[:, :ncols], rope_t[:, 0, :ncols])
                        t2 = wkpool.tile([P, SEQ], FP32, tag="t2")
                        H2 = HD // 2
                        nc.vector.tensor_mul(t2[0:H2, :ncols], ps[H2:P, :ncols], rope_t[0:H2, 1, :ncols])
                        nc.vector.tensor_mul(t2[H2:P, :ncols], ps[0:H2, :ncols], rope_t[H2:P, 1, :ncols])
                        nc.vector.tensor_add(dst[:, h, :ncols], t1[:, :ncols], t2[:, :ncols])

                srckv = xb if nkv == SEQ else xkvb
                for vc in range(vch):  # v, natural layout, compacted tokens
                    pc = min(P, nkv - vc * P)
                    ps = pA.tile([P, SEQ], FP32, tag="pa")
                    for kc in range(KC):
                        nc.tensor.matmul(
                            ps[0:pc, :HF], srckv[:, kc, vc * P:vc * P + pc], wv_s[:, kc, :],
                            start=(kc == 0), stop=(kc == KC - 1),
                        )
                    nc.vector.tensor_copy(v_b[0:pc, vc, :], ps[0:pc, :HF])

                # ---- phase B: attention, software-pipelined per head ----
                # Chunk list: 4 history chunks (s0=0) then compacted new
                # chunks; new chunk at compact offset j0 only reaches queries
                # s >= j0 (orig position >= compact index), so narrow ops.
                # The PE queue is strict FIFO, so the emission order below IS
                # the execution order: rowsum/PV for chunk ci-2 are emitted
                # between score matmuls so the PE never heads-of-line blocks
                # on the exp->gate chain, and o-proj groups of the PREVIOUS
                # batch are sprinkled in as dense filler.
                aT = apool.tile([P, HPC, SEQ], BF16D)
                ounits = list(range(16)) if prev_o is not None else []
                for h in range(HPC):
                    chunks = []
                    for t in range(SC):
                        chunks.append((kThb[:, h, t * P:(t + 1) * P],
                                       vhb[:, h * SC + t, :], P, 0))
                    for tcn_i in range(tcn):
                        j0 = tcn_i * P
                        pc = min(P, nkv - j0)
                        s0 = int(newpos[b][j0]) if j0 < len(newpos[b]) else SEQ - 1
                        chunks.append((kT_b[:, h, j0:j0 + pc],
                                       v_b[0:pc, tcn_i, h * P:(h + 1) * P], pc, s0))
                    ee = epool.tile([P, nchm, SEQ], BF16D)
                    pr = pR.tile([1, SEQ], FP32, tag="pr")
                    po = pOT.tile([P, SEQ], FP32, tag="po")

                    def rs_pv(ci):
                        _, vlhs, pc, s0 = chunks[ci]
                        nc.tensor.matmul(
                            pr[:, s0:], ones_bf[0:pc, :], ee[0:pc, ci, s0:],
                            start=(ci == 0), stop=(ci == nch - 1),
                            skip_group_check=True,
                        )
                        nc.tensor.matmul(
                            po[:, s0:], vlhs, ee[0:pc, ci, s0:],
                            start=(ci == 0), stop=(ci == nch - 1),
                            skip_group_check=True,
                        )

                    for ci, (klhs, vlhs, pc, s0) in enumerate(chunks):
                        pscr = pS.tile([P, SEQ], FP32, tag="ps")
                        nc.tensor.matmul(pscr[0:pc, s0:], klhs, qT_b[:, h, s0:], start=True, stop=True)
                        # exp in-place in PSUM: ScalarE's fast port is PSUM on
                        # both sides; ACT->SBUF measured ~8x slower.
                        nc.scalar.activation(pscr[0:pc, s0:], pscr[0:pc, s0:], mybir.ActivationFunctionType.Exp)
                        nc.vector.tensor_mul(ee[0:pc, ci, s0:], pscr[0:pc, s0:], gtb[0:pc, ci, s0:])
                        if ci >= 2:
                            rs_pv(ci - 2)
                            if ci % 2 == 0 and ounits:
                                oproj_unit(prev_o[0], prev_o[1], ounits.pop(0))
                    rs_pv(nch - 2)
                    if ounits:
                        oproj_unit(prev_o[0], prev_o[1], ounits.pop(0))
                    rs_pv(nch - 1)
                    rinv = smpool.tile([1, SEQ], FP32, tag="rinv")
                    nc.vector.reciprocal_approx_fast(rinv[:], pr[:])
                    rb_s = smpool.tile([P, SEQ], FP32, tag="rbs")
                    nc.gpsimd.partition_broadcast(rb_s[:], rinv[:])
                    nc.vector.tensor_mul(aT[:, h, :], po[:], rb_s[:])
                # drain any o-proj groups this batch's slots didn't absorb
                while ounits:
                    oproj_unit(prev_o[0], prev_o[1], ounits.pop(0))
                prev_o = (aT, ts)

            # final batch's o-projection (no next attention phase to hide in)
            for k in range(16):
                oproj_unit(prev_o[0], prev_o[1], k)
    nc.finalize()
    return nc


_CACHE = {}


def _get_program(nkvs, newpos):
    key = (nkvs, tuple(tuple(int(v) for v in p) for p in newpos))
    if key not in _CACHE:
        try:
            _CACHE[key] = build_program(nkvs, newpos)
        except ValueError:
            # SBUF pressure (large nkv): shallower weight prefetch
            _CACHE[key] = build_program(nkvs, newpos, wu_bufs=2)
    return _CACHE[key]


def _prep_inputs(inputs):
    x = np.asarray(inputs["x"], np.float32)
    router = np.asarray(inputs["router"], np.float32)
    cache_k = np.asarray(inputs["cache_k"], np.float32)
    cache_v = np.asarray(inputs["cache_v"], np.float32)
    cache_mask = np.asarray(inputs["cache_mask"])
    mask = np.asarray(inputs["mask"], np.float32)
    wq = np.asarray(inputs["wq"], np.float32)
    wk = np.asarray(inputs["wk"], np.float32)
    wv = np.asarray(inputs["wv"], np.float32)
    wo = np.asarray(inputs["wo"], np.float32)
    position_ids = np.asarray(inputs["position_ids"], np.int64)
    batch_exec = np.asarray(inputs["batch_exec"], np.int64)
    start_pos = int(inputs["start_pos"])
    assert start_pos == START and x.shape == (BSZ, SEQ, DIM)

    # compacted new-token index lists (order-preserving)
    pen_new = router[:, :, 0] != 0.0                                  # [8, 512]
    idx = [np.nonzero(pen_new[b])[0] for b in range(BSZ)]
    # compaction stops paying (and SBUF stops fitting) for dense batches:
    # above 384 usable tokens run uncompacted in original token order, with
    # the router folded into the per-element gate like the history chunks
    idx = [i if len(i) <= 384 else np.arange(SEQ) for i in idx]
    nkvs = tuple(max(16, ((len(i) + 15) // 16) * 16) for i in idx)
    nkv = max(nkvs)
    tcn = (nkv + P - 1) // P
    nch = SC + tcn

    # x packed per batch into the SBUF tile layout [b, p, kc, tok]
    xT = x.reshape(BSZ, SEQ, KC, P)               # tok-major view of features
    xp = np.ascontiguousarray(xT.transpose(0, 3, 2, 1)).astype(BF16)  # [8,128,32,512]
    nkvc = max([n for n in nkvs if n < SEQ], default=16)
    xsel = np.zeros((BSZ, nkvc, KC, P), np.float32)
    for b in range(BSZ):
        if nkvs[b] < SEQ:
            xsel[b, :len(idx[b])] = xT[b, idx[b]]
    xkvp = np.ascontiguousarray(xsel.transpose(0, 3, 2, 1)).astype(BF16)

    # RoPE tables gathered at position_ids, packed [b, p(hd), table, tok]
    inv_freq = 1.0 / (ROPE_BASE ** (np.arange(0, HD, 2, dtype=np.float32) / HD))
    t = np.arange(KV, dtype=np.float32)
    emb = np.concatenate([t[:, None] * inv_freq, t[:, None] * inv_freq], axis=-1)
    cos_t = np.cos(emb).astype(np.float32)[position_ids]   # [8, 512, 128]
    sin_t = np.sin(emb).astype(np.float32)[position_ids]
    sign = np.where(np.arange(HD) < HD // 2, -1.0, 1.0).astype(np.float32)
    scale = np.float32(1.0 / np.sqrt(HD))
    ropeq = np.stack([cos_t * scale, (sin_t * sign) * scale], axis=1)  # [8,2,512,128]
    ropeqp = np.ascontiguousarray(ropeq.transpose(0, 3, 1, 2)).astype(np.float32)
    ropek = np.zeros((BSZ, 2, nkv, HD), np.float32)
    for b in range(BSZ):
        nb = len(idx[b])
        ropek[b, 0, :nb] = cos_t[b, idx[b]]
        ropek[b, 1, :nb] = sin_t[b, idx[b]] * sign
    ropekp = np.ascontiguousarray(ropek.transpose(0, 3, 1, 2)).astype(np.float32)

    # history cache slices (host-side gather = sharding)
    k_hist = cache_k[batch_exec, :, :START, :]   # [8, 32, 512, 128]
    v_hist = cache_v[batch_exec, :, :START, :]

    # multiplicative 0/1 gate: causal AND cache-usable, with the new-token
    # half compacted to idx[b]; packed [b, p, chunk, s]
    pen_hist = cache_mask[batch_exec, :START].astype(bool)            # [8, 512]
    causal_ok = (mask[0, 0] > -0.5)                                   # [512 s, 1024 t]
    gate_hist = causal_ok.T[None, :START, :] & pen_hist[:, :, None]   # [8, 512, 512]
    gate_new = np.zeros((BSZ, nkv, SEQ), bool)
    for b in range(BSZ):
        nb = len(idx[b])
        gate_new[b, :nb] = causal_ok.T[START + idx[b], :] & pen_new[b, idx[b]][:, None]
    gate = np.concatenate([gate_hist, gate_new], axis=1)              # [8, 512+nkv, 512]
    pad = nch * P - gate.shape[1]
    if pad:
        gate = np.concatenate([gate, np.zeros((BSZ, pad, SEQ), bool)], axis=1)
    gatep = np.ascontiguousarray(
        gate.reshape(BSZ, nch, P, SEQ).transpose(0, 2, 1, 3)
        .astype(np.float32)).astype(BF16)                             # [8,128,nch,512]

    in_maps = []
    for c in range(NC):
        hs, he = c * HPC, (c + 1) * HPC
        fs, fe = c * HF, (c + 1) * HF
        # q/k units [2*HPC, p, kc, 128]: unit (proj, h) = W[fs+h*128 : ...].T
        wqkT = np.stack([w[fs:fe].T for w in (wq, wk)])   # [2, 4096, 512]
        wqk = (wqkT.reshape(2, KC, P, HPC, HD).transpose(0, 3, 2, 1, 4)
               .reshape(2 * HPC, P, KC, HD))
        wvT = wv[fs:fe].T                                  # [4096, 512]
        wvp = wvT.reshape(KC, P, HF).transpose(1, 0, 2)    # [128, 32, 512]
        woTc = wo[:, fs:fe].T                              # [512, 4096]
        wop = woTc.reshape(HPC, P, DIM).transpose(1, 0, 2) # [128, 4, 4096]
        kThp = k_hist[:, hs:he].transpose(0, 3, 1, 2)      # [8, 128hd, 4h, 512]
        vhp = (v_hist[:, hs:he].reshape(BSZ, HPC, SC, P, HD)
               .transpose(0, 3, 1, 2, 4).reshape(BSZ, P, HPC * SC, HD))
        in_maps.append({
            "xp": xp,
            "xkvp": xkvp,
            "wqk": np.ascontiguousarray(wqk).astype(BF16),
            "wvp": np.ascontiguousarray(wvp).astype(BF16),
            "wop": np.ascontiguousarray(wop).astype(BF16),
            "ropeq": ropeqp,
            "ropek": ropekp,
            "kThp": np.ascontiguousarray(kThp).astype(BF16),
            "vhp": np.ascontiguousarray(vhp).astype(BF16),
            "gatep": gatep,
        })
    return in_maps, nkvs, idx


def _install_profile_hook():
    """The agent image's antenv lacks axon_hooks; shim it so trace=True works."""
    import sys, types
    if "antenv.axon_hooks" in sys.modules:
        return
    try:
        from trn_agent_boot.trn_boot import _ntff_profile_via_ctypes
    except ImportError:
        return
    mod = types.ModuleType("antenv.axon_hooks")
    mod._hook = _ntff_profile_via_ctypes("/opt/axon/libaxon_pjrt.so")

    def set_axon_ntff_profile_hook(h):
        mod._hook = h

    def get_axon_ntff_profile_hook():
        return mod._hook

    mod.set_axon_ntff_profile_hook = set_axon_ntff_profile_hook
    mod.get_axon_ntff_profile_hook = get_axon_ntff_profile_hook
    sys.modules["antenv.axon_hooks"] = mod
    import antenv
    antenv.axon_hooks = mod


def _run(inputs, trace=False):
    if trace:
        _install_profile_hook()
    in_maps, nkvs, newpos = _prep_inputs(inputs)
    nc = _get_program(nkvs, newpos)
    res = run_bass_kernel_spmd(nc, in_maps, core_ids=list(range(NC)), trace=trace)
    acc = np.zeros((BSZ * SEQ, DIM), np.float32)
    for c in range(NC):
        acc += res.results[c]["out"].astype(np.float32)
    return acc.reshape(BSZ, SEQ, DIM), res


def kernel(**inputs):
    out, _ = _run(inputs, trace=False)
    return out


# revision 19
# speedup vs baseline: 1.1586x; 1.0004x over previous
"""Trainium2 Bass kernel: attention layer with KV cache, tensor-parallel over heads.

Sharding (8 NeuronCores): Megatron-style TP over the 32 heads -> 4 heads/core.
  - wq/wk/wv: column-parallel (each core owns a [512, 4096] output shard)
  - wo: row-parallel (each core owns wo[:, c*512:(c+1)*512]); cores emit
    partial o-proj outputs which the host sums (RowParallel unshard).
  - cache_k/cache_v: sharded along the head axis; history rows/positions are
    gathered host-side from batch_exec/start_pos (pure indexing).

Device layout trick: Q/K are projected directly in transposed [head_dim, tok]
layout (head dim = PSUM partitions), V in natural [tok, head_dim] layout, so
scores^T, PV, and the o-projection all consume each other's outputs as
matmul operands with zero on-device transposes.  Softmax runs without the
max-subtraction (scores are O(1), exp is safe in f32) so the kv-axis
(partition-axis) row-sum comes from a ones-vector matmul; normalization is a
rank-1 broadcast matmul of 1/r.  exp runs IN-PLACE in PSUM: the ACT engine's
fast port is PSUM on both sides (the ACT->SBUF write path measured ~8x
slower per element).

New-token compaction: router[:, :, 0] gates ~half the new cache positions
off for every query, so their K/V projections and score/PV columns are dead
work.  The host compacts each batch's unmasked new tokens (order preserved)
and the program projects/attends over pad16(count_b) columns per batch,
with exact causal narrowing from the compacted token positions (chunk at
compact offset j0 only reaches queries s >= orig_pos(j0)).  Batches with
>384 usable tokens fall back to uncompacted identity order (SBUF would not
fit the staging buffer, and compaction stops paying).  The program is built
per-(counts, positions) and cached, so repeated calls with the same router
compile once.

Scheduling: the PE queue is strict FIFO, so emission order is execution
order.  Attention is software-pipelined per head (rowsum/PV matmuls lag two
chunks behind the score matmuls), and the PREVIOUS batch's o-projection
groups are interleaved into the attention beats as dense PE filler, so the
PE never head-of-line blocks on the exp->gate chain.
"""

import numpy as np
import ml_dtypes

import concourse.bass as bass
import concourse.bacc as bacc
import concourse.tile as tile
from concourse import mybir
from concourse.bass_utils import run_bass_kernel_spmd

BF16 = np.dtype(ml_dtypes.bfloat16)

# Problem shape (hardcoded per the task contract)
BSZ = 8
SEQ = 512
DIM = 4096
NH = 32
HD = 128
START = 512
KV = START + SEQ          # 1024
NC = 8                    # cores
HPC = NH // NC            # 4 heads per core
HF = HPC * HD             # 512 local features
P = 128
KC = DIM // P             # 32 contraction chunks
SC = SEQ // P             # 4 seq chunks (also history kv chunks)
ROPE_BASE = 10000.0

FP32 = mybir.dt.float32
BF16D = mybir.dt.bfloat16


def build_program(nkvs, newpos, wu_bufs=4):
    # Every DRAM parameter is pre-packed host-side into the exact SBUF tile
    # layout (partition-major), so each DMA is a contiguous >=8KB-per-partition
    # stream: ~128 fat descriptors instead of thousands of 256B ones.
    # nkvs: per-batch padded compacted new-token counts (multiples of 16).
    # newpos[b][j] = original position of compacted token j (for exact causal
    # narrowing): new chunk starting at compact j0 only reaches queries
    # s >= newpos[b][j0].
    nkvm = max(nkvs)
    tcnm = (nkvm + P - 1) // P        # max new-token kv chunks
    nchm = SC + tcnm                  # max total kv chunks per head
    vchm = (nkvm + P - 1) // P        # max v chunks (tokens on partitions)
    # batches with nkv == SEQ run uncompacted straight from the resident x
    # tile; the compacted-x staging buffer only needs the compacted batches
    nkvc = max([n for n in nkvs if n < SEQ], default=16)

    nc = bacc.Bacc(None, target_bir_lowering=False)
    x_d = nc.declare_dram_parameter("xp", [BSZ, P, KC, SEQ], BF16D, isOutput=False)
    xkv_d = nc.declare_dram_parameter("xkvp", [BSZ, P, KC, nkvc], BF16D, isOutput=False)
    wqk_d = nc.declare_dram_parameter("wqk", [2 * HPC, P, KC, P], BF16D, isOutput=False)
    wv_d = nc.declare_dram_parameter("wvp", [P, KC, HF], BF16D, isOutput=False)
    woT_d = nc.declare_dram_parameter("wop", [P, HPC, DIM], BF16D, isOutput=False)
    ropeq_d = nc.declare_dram_parameter("ropeq", [BSZ, P, 2, SEQ], FP32, isOutput=False)
    ropek_d = nc.declare_dram_parameter("ropek", [BSZ, P, 2, nkvm], FP32, isOutput=False)
    kTh_d = nc.declare_dram_parameter("kThp", [BSZ, P, HPC, START], BF16D, isOutput=False)
    vh_d = nc.declare_dram_parameter("vhp", [BSZ, P, HPC * SC, HD], BF16D, isOutput=False)
    gateT_d = nc.declare_dram_parameter("gatep", [BSZ, P, nchm, SEQ], BF16D, isOutput=False)
    out_d = nc.declare_dram_parameter("out", [BSZ * SEQ, DIM], BF16D, isOutput=True)

    from contextlib import ExitStack

    with ExitStack() as ctx:
        tc = ctx.enter_context(tile.TileContext(nc))
        cpool = ctx.enter_context(tc.tile_pool(name="const", bufs=1))
        wupool = ctx.enter_context(tc.tile_pool(name="wu", bufs=wu_bufs))
        xpool = ctx.enter_context(tc.tile_pool(name="xb", bufs=1))
        rpool = ctx.enter_context(tc.tile_pool(name="rope", bufs=1))
        qkvpool = ctx.enter_context(tc.tile_pool(name="qkv", bufs=2))
        hpool = ctx.enter_context(tc.tile_pool(name="hist", bufs=1))
        wkpool = ctx.enter_context(tc.tile_pool(name="work", bufs=2))
        epool = ctx.enter_context(tc.tile_pool(name="ee", bufs=2))
        apool = ctx.enter_context(tc.tile_pool(name="at", bufs=2))
        smpool = ctx.enter_context(tc.tile_pool(name="small", bufs=1))
        gpool = ctx.enter_context(tc.tile_pool(name="gate", bufs=1))
        popool = ctx.enter_context(tc.tile_pool(name="po", bufs=2))
        pA = ctx.enter_context(tc.tile_pool(name="pA", bufs=2, space="PSUM"))
        pS = ctx.enter_context(tc.tile_pool(name="pS", bufs=2, space="PSUM"))
        pR = ctx.enter_context(tc.tile_pool(name="pR", bufs=1, space="PSUM"))
        pOT = ctx.enter_context(tc.tile_pool(name="pOT", bufs=1, space="PSUM"))
        pP = ctx.enter_context(tc.tile_pool(name="pP", bufs=2, space="PSUM"))
        if True:
            # ---- constants (weights emitted after b0's hot DMAs, see below) ----
            ones_bf = cpool.tile([P, 1], BF16D)
            nc.gpsimd.memset(ones_bf[:], 1.0)
            wv_s = cpool.tile([P, KC, HF], BF16D)
            woT_s = cpool.tile([P, HPC, DIM], BF16D)

            def oproj_unit(aT_prev, ts_prev, k):
                # one (seq-chunk, out-chunk-pair) group of the previous
                # batch's o-projection: dense PE filler between the
                # exp/gate-gated attention beats.
                sc4, og = divmod(k, 4)
                pout = popool.tile([P, 2, HF], BF16D, tag="pout")
                for oi in range(2):
                    oc = og * 2 + oi
                    pp = pP.tile([P, HF], FP32, tag="pp")
                    for h in range(HPC):
                        nc.tensor.matmul(
                            pp[:],
                            aT_prev[:, h, sc4 * P:(sc4 + 1) * P],
                            woT_s[:, h, oc * HF:(oc + 1) * HF],
                            start=(h == 0), stop=(h == HPC - 1),
                        )
                    nc.vector.tensor_copy(pout[:, oi, :], pp[:])
                nc.gpsimd.dma_start(
                    out_d[ts_prev + sc4 * P: ts_prev + (sc4 + 1) * P,
                          og * 2 * HF:(og * 2 + 2) * HF],
                    pout[:],
                )

            prev_o = None  # (aT, ts) of the batch whose o-proj is pending
            for b in range(BSZ):
                ts = b * SEQ
                nkv = nkvs[b]
                tcn = (nkv + P - 1) // P
                nch = SC + tcn
                vch = (nkv + P - 1) // P
                # hoist the first weight unit ahead of the 4MB x stream so
                # the next phase-A matmul group starts ~1.5MB into the DMA
                wuf0 = wupool.tile([P, 16, P], BF16D, tag="wu")
                wuf1 = wupool.tile([P, 16, P], BF16D, tag="wu")
                wu_first = [wuf0, wuf1]
                # at program start the scalar DGE queue is empty: split the
                # first weight unit across both queues so the first matmul
                # group starts ~0.25MB+1MB into the sync stream (b>0 keeps
                # everything on sync -- the scalar queue carries the rope/
                # history/gate tensors there and must not be delayed)
                eng1 = nc.scalar if b == 0 else nc.sync
                for wc in range(4):
                    eng = (nc.sync, nc.sync, eng1, eng1)[wc]
                    eng.dma_start(wu_first[wc // 2][:, (wc % 2) * 8:(wc % 2 + 1) * 8, :],
                                  wqk_d[0, :, wc * 8:(wc + 1) * 8, :])
                xb = xpool.tile([P, KC, SEQ], BF16D)
                for xc in range(4):  # split so the first matmuls start early
                    nc.sync.dma_start(
                        xb[:, xc * 8:(xc + 1) * 8, :],
                        x_d[b, :, xc * 8:(xc + 1) * 8, :],
                    )
                xkvb = xpool.tile([P, KC, nkvc], BF16D, tag="xkv")
                # small hot tensors ride the scalar engine's DGE queues so
                # they are not stuck behind the multi-MB sync-queue streams
                ropeq_b = rpool.tile([P, 2, SEQ], FP32, tag="rq")
                nc.scalar.dma_start(ropeq_b[:], ropeq_d[b])
                ropek_b = rpool.tile([P, 2, nkvm], FP32, tag="rk")
                nc.scalar.dma_start(ropek_b[:, :, :nkv], ropek_d[b, :, :, :nkv])
                kThb = hpool.tile([P, HPC, START], BF16D)
                nc.scalar.dma_start(kThb[:], kTh_d[b])
                vhb = hpool.tile([P, HPC * SC, HD], BF16D)
                nc.scalar.dma_start(vhb[:], vh_d[b])
                gtb = gpool.tile([P, nchm, SEQ], BF16D)
                nc.scalar.dma_start(gtb[:, :nch, :], gateT_d[b, :, :nch, :])
                if b == 0:
                    nc.scalar.dma_start(woT_s[:, 0:2, :], woT_d[:, 0:2, :])
                    nc.scalar.dma_start(woT_s[:, 2:4, :], woT_d[:, 2:4, :])

                # ---- phase A: QKV projections (+RoPE for q/k) ----
                qT_b = qkvpool.tile([P, HPC, SEQ], BF16D, tag="qT")
                kT_b = qkvpool.tile([P, HPC, nkvm], BF16D, tag="kT")
                v_b = qkvpool.tile([P, vchm, HF], BF16D, tag="v", bufs=1)

                for proj in range(2):  # 0=q (full tokens), 1=k (compacted)
                    if proj == 1 and nkv < SEQ:
                        # xkv is only needed by the k/v units; emitting its
                        # DMA here keeps the q weight-units at the head of
                        # the sync queue (NOTE: the scalar DGE queue is
                        # bandwidth-limited -- bulk streams belong on sync)
                        for xc in range(2):
                            nc.sync.dma_start(
                                xkvb[:, xc * 16:(xc + 1) * 16, :nkv],
                                xkv_d[b, :, xc * 16:(xc + 1) * 16, :nkv],
                            )
                    if b == 0 and proj == 1:
                        # consts queue on the sync FIFO only after b0's
                        # q-units, so the critical startup stream drains first
                        nc.sync.dma_start(wv_s[:, 0:16, :], wv_d[:, 0:16, :])
                        nc.sync.dma_start(wv_s[:, 16:32, :], wv_d[:, 16:32, :])
                    if proj == 0:
                        dst, src, rope_t, ncols = qT_b, xb, ropeq_b, SEQ
                    else:
                        src = xb if nkv == SEQ else xkvb
                        dst, rope_t, ncols = kT_b, ropek_b, nkv
                    for h in range(HPC):
                        if proj == 0 and h == 0:
                            wu = wu_first
                        else:
                            wu0 = wupool.tile([P, 16, P], BF16D, tag="wu")
                            wu1 = wupool.tile([P, 16, P], BF16D, tag="wu")
                            wu = [wu0, wu1]
                            for half in range(2):
                                u = proj * HPC + h
                                nc.sync.dma_start(wu[half][:, 0:8, :], wqk_d[u, :, half * 16:half * 16 + 8, :])
                                nc.sync.dma_start(wu[half][:, 8:16, :], wqk_d[u, :, half * 16 + 8:half * 16 + 16, :])
                        ps = pA.tile([P, SEQ], FP32, tag="pa")
                        for kc in range(KC):
                            nc.tensor.matmul(
                                ps[:, :ncols], wu[kc // 16][:, kc % 16, :], src[:, kc, :ncols],
                                start=(kc == 0), stop=(kc == KC - 1),
                            )
                        # RoPE: dst = ps*cos + shift64(ps)*sin_signed
                        t1 = wkpool.tile([P, SEQ], FP32, tag="t1")
                        nc.vector.tensor_mul(t1[:, :ncols], ps[:, :ncols], rope_t[:, 0, :ncols])
                        t2 = wkpool.tile([P, SEQ], FP32, tag="t2")
                        H2 = HD // 2
                        nc.vector.tensor_mul(t2[0:H2, :ncols], ps[H2:P, :ncols], rope_t[0:H2, 1, :ncols])
                        nc.vector.tensor_mul(t2[H2:P, :ncols], ps[0:H2, :ncols], rope_t[H2:P, 1, :ncols])
                        nc.vector.tensor_add(dst[:, h, :ncols], t1[:, :ncols], t2[:, :ncols])

                srckv = xb if nkv == SEQ else xkvb
                for vc in range(vch):  # v, natural layout, compacted tokens
                    pc = min(P, nkv - vc * P)
                    ps = pA.tile([P, SEQ], FP32, tag="pa")
                    for kc in range(KC):
                        nc.tensor.matmul(
                            ps[0:pc, :HF], srckv[:, kc, vc * P:vc * P + pc], wv_s[:, kc, :],
                            start=(kc == 0), stop=(kc == KC - 1),
                        )
                    nc.vector.tensor_copy(v_b[0:pc, vc, :], ps[0:pc, :HF])

                # ---- phase B: attention, software-pipelined per head ----
                # Chunk list: 4 history chunks (s0=0) then compacted new
                # chunks; new chunk at compact offset j0 only reaches queries
                # s >= j0 (orig position >= compact index), so narrow ops.
                # The PE queue is strict FIFO, so the emission order below IS
                # the execution order: rowsum/PV for chunk ci-2 are emitted
                # between score matmuls so the PE never heads-of-line blocks
                # on the exp->gate chain, and o-proj groups of the PREVIOUS
                # batch are sprinkled in as dense filler.
                aT = apool.tile([P, HPC, SEQ], BF16D)
                ounits = list(range(16)) if prev_o is not None else []
                for h in range(HPC):
                    chunks = []
                    for t in range(SC):
                        chunks.append((kThb[:, h, t * P:(t + 1) * P],
                                       vhb[:, h * SC + t, :], P, 0))
                    for tcn_i in range(tcn):
                        j0 = tcn_i * P
                        pc = min(P, nkv - j0)
                        s0 = int(newpos[b][j0]) if j0 < len(newpos[b]) else SEQ - 1
                        chunks.append((kT_b[:, h, j0:j0 + pc],
                                       v_b[0:pc, tcn_i, h * P:(h + 1) * P], pc, s0))
                    ee = epool.tile([P, nchm, SEQ], BF16D)
                    pr = pR.tile([1, SEQ], FP32, tag="pr")
                    po = pOT.tile([P, SEQ], FP32, tag="po")

                    def rs_pv(ci):
                        _, vlhs, pc, s0 = chunks[ci]
                        nc.tensor.matmul(
                            pr[:, s0:], ones_bf[0:pc, :], ee[0:pc, ci, s0:],
                            start=(ci == 0), stop=(ci == nch - 1),
                            skip_group_check=True,
                        )
                        nc.tensor.matmul(
                            po[:, s0:], vlhs, ee[0:pc, ci, s0:],
                            start=(ci == 0), stop=(ci == nch - 1),
                            skip_group_check=True,
                        )

                    for ci, (klhs, vlhs, pc, s0) in enumerate(chunks):
                        pscr = pS.tile([P, SEQ], FP32, tag="ps")
                        nc.tensor.matmul(pscr[0:pc, s0:], klhs, qT_b[:, h, s0:], start=True, stop=True)
                        # exp in-place in PSUM: ScalarE's fast port is PSUM on
                        # both sides; ACT->SBUF measured ~8x slower.
                        nc.scalar.activation(pscr[0:pc, s0:], pscr[0:pc, s0:], mybir.ActivationFunctionType.Exp)
                        nc.vector.tensor_mul(ee[0:pc, ci, s0:], pscr[0:pc, s0:], gtb[0:pc, ci, s0:])
                        if ci >= 2:
                            rs_pv(ci - 2)
                            if ci % 2 == 0 and ounits:
                                oproj_unit(prev_o[0], prev_o[1], ounits.pop(0))
                    rs_pv(nch - 2)
                    if ounits:
                        oproj_unit(prev_o[0], prev_o[1], ounits.pop(0))
                    rs_pv(nch - 1)
                    rinv = smpool.tile([1, SEQ], FP32, tag="rinv")
                    nc.vector.reciprocal_approx_fast(rinv[:], pr[:])
                    rb_s = smpool.tile([P, SEQ], FP32, tag="rbs")
                    nc.gpsimd.partition_broadcast(rb_s[:], rinv[:])
                    nc.vector.tensor_mul(aT[:, h, :], po[:], rb_s[:])
                # drain any o-proj groups this batch's slots didn't absorb
                while ounits:
                    oproj_unit(prev_o[0], prev_o[1], ounits.pop(0))
                prev_o = (aT, ts)

            # final batch's o-projection (no next attention phase to hide in)
            for k in range(16):
                oproj_unit(prev_o[0], prev_o[1], k)
    nc.finalize()
    return nc


_CACHE = {}


def _get_program(nkvs, newpos):
    key = (nkvs, tuple(tuple(int(v) for v in p) for p in newpos))
    if key not in _CACHE:
        try:
            _CACHE[key] = build_program(nkvs, newpos)
        except ValueError:
            # SBUF pressure (large nkv): shallower weight prefetch
            _CACHE[key] = build_program(nkvs, newpos, wu_bufs=2)
    return _CACHE[key]


def _prep_inputs(inputs):
    x = np.asarray(inputs["x"], np.float32)
    router = np.asarray(inputs["router"], np.float32)
    cache_k = np.asarray(inputs["cache_k"], np.float32)
    cache_v = np.asarray(inputs["cache_v"], np.float32)
    cache_mask = np.asarray(inputs["cache_mask"])
    mask = np.asarray(inputs["mask"], np.float32)
    wq = np.asarray(inputs["wq"], np.float32)
    wk = np.asarray(inputs["wk"], np.float32)
    wv = np.asarray(inputs["wv"], np.float32)
    wo = np.asarray(inputs["wo"], np.float32)
    position_ids = np.asarray(inputs["position_ids"], np.int64)
    batch_exec = np.asarray(inputs["batch_exec"], np.int64)
    start_pos = int(inputs["start_pos"])
    assert start_pos == START and x.shape == (BSZ, SEQ, DIM)

    # compacted new-token index lists (order-preserving)
    pen_new = router[:, :, 0] != 0.0                                  # [8, 512]
    idx = [np.nonzero(pen_new[b])[0] for b in range(BSZ)]
    # compaction stops paying (and SBUF stops fitting) for dense batches:
    # above 384 usable tokens run uncompacted in original token order, with
    # the router folded into the per-element gate like the history chunks
    idx = [i if len(i) <= 384 else np.arange(SEQ) for i in idx]
    nkvs = tuple(max(16, ((len(i) + 15) // 16) * 16) for i in idx)
    nkv = max(nkvs)
    tcn = (nkv + P - 1) // P
    nch = SC + tcn

    # x packed per batch into the SBUF tile layout [b, p, kc, tok]
    xT = x.reshape(BSZ, SEQ, KC, P)               # tok-major view of features
    xp = np.ascontiguousarray(xT.transpose(0, 3, 2, 1)).astype(BF16)  # [8,128,32,512]
    nkvc = max([n for n in nkvs if n < SEQ], default=16)
    xsel = np.zeros((BSZ, nkvc, KC, P), np.float32)
    for b in range(BSZ):
        if nkvs[b] < SEQ:
            xsel[b, :len(idx[b])] = xT[b, idx[b]]
    xkvp = np.ascontiguousarray(xsel.transpose(0, 3, 2, 1)).astype(BF16)

    # RoPE tables gathered at position_ids, packed [b, p(hd), table, tok]
    inv_freq = 1.0 / (ROPE_BASE ** (np.arange(0, HD, 2, dtype=np.float32) / HD))
    t = np.arange(KV, dtype=np.float32)
    emb = np.concatenate([t[:, None] * inv_freq, t[:, None] * inv_freq], axis=-1)
    cos_t = np.cos(emb).astype(np.float32)[position_ids]   # [8, 512, 128]
    sin_t = np.sin(emb).astype(np.float32)[position_ids]
    sign = np.where(np.arange(HD) < HD // 2, -1.0, 1.0).astype(np.float32)
    scale = np.float32(1.0 / np.sqrt(HD))
    ropeq = np.stack([cos_t * scale, (sin_t * sign) * scale], axis=1)  # [8,2,512,128]
    ropeqp = np.ascontiguousarray(ropeq.transpose(0, 3, 1, 2)).astype(np.float32)
    ropek = np.zeros((BSZ, 2, nkv, HD), np.float32)
    for b in range(BSZ):
        nb = len(idx[b])
        ropek[b, 0, :nb] = cos_t[b, idx[b]]
        ropek[b, 1, :nb] = sin_t[b, idx[b]] * sign
    ropekp = np.ascontiguousarray(ropek.transpose(0, 3, 1, 2)).astype(np.float32)

    # history cache slices (host-side gather = sharding)
    k_hist = cache_k[batch_exec, :, :START, :]   # [8, 32, 512, 128]
    v_hist = cache_v[batch_exec, :, :START, :]

    # multiplicative 0/1 gate: causal AND cache-usable, with the new-token
    # half compacted to idx[b]; packed [b, p, chunk, s]
    pen_hist = cache_mask[batch_exec, :START].astype(bool)            # [8, 512]
    causal_ok = (mask[0, 0] > -0.5)                                   # [512 s, 1024 t]
    gate_hist = causal_ok.T[None, :START, :] & pen_hist[:, :, None]   # [8, 512, 512]
    gate_new = np.zeros((BSZ, nkv, SEQ), bool)
    for b in range(BSZ):
        nb = len(idx[b])
        gate_new[b, :nb] = causal_ok.T[START + idx[b], :] & pen_new[b, idx[b]][:, None]
    gate = np.concatenate([gate_hist, gate_new], axis=1)              # [8, 512+nkv, 512]
    pad = nch * P - gate.shape[1]
    if pad:
        gate = np.concatenate([gate, np.zeros((BSZ, pad, SEQ), bool)], axis=1)
    gatep = np.ascontiguousarray(
        gate.reshape(BSZ, nch, P, SEQ).transpose(0, 2, 1, 3)
        .astype(np.float32)).astype(BF16)                             # [8,128,nch,512]

    in_maps = []
    for c in range(NC):
        hs, he = c * HPC, (c + 1) * HPC
        fs, fe = c * HF, (c + 1) * HF
        # q/k units [2*HPC, p, kc, 128]: unit (proj, h) = W[fs+h*128 : ...].T
        wqkT = np.stack([w[fs:fe].T for w in (wq, wk)])   # [2, 4096, 512]
        wqk = (wqkT.reshape(2, KC, P, HPC, HD).transpose(0, 3, 2, 1, 4)
               .reshape(2 * HPC, P, KC, HD))
        wvT = wv[fs:fe].T                                  # [4096, 512]
        wvp = wvT.reshape(KC, P, HF).transpose(1, 0, 2)    # [128, 32, 512]
        woTc = wo[:, fs:fe].T                              # [512, 4096]
        wop = woTc.reshape(HPC, P, DIM).transpose(1, 0, 2) # [128, 4, 4096]
        kThp = k_hist[:, hs:he].transpose(0, 3, 1, 2)      # [8, 128hd, 4h, 512]
        vhp = (v_hist[:, hs:he].reshape(BSZ, HPC, SC, P, HD)
               .transpose(0, 3, 1, 2, 4).reshape(BSZ, P, HPC * SC, HD))
        in_maps.append({
            "xp": xp,
            "xkvp": xkvp,
            "wqk": np.ascontiguousarray(wqk).astype(BF16),
            "wvp": np.ascontiguousarray(wvp).astype(BF16),
            "wop": np.ascontiguousarray(wop).astype(BF16),
            "ropeq": ropeqp,
            "ropek": ropekp,
            "kThp": np.ascontiguousarray(kThp).astype(BF16),
            "vhp": np.ascontiguousarray(vhp).astype(BF16),
            "gatep": gatep,
        })
    return in_maps, nkvs, idx


def _install_profile_hook():
    """The agent image's antenv lacks axon_hooks; shim it so trace=True works."""
    import sys, types
    if "antenv.axon_hooks" in sys.modules:
        return
    try:
        from trn_agent_boot.trn_boot import _ntff_profile_via_ctypes
    except ImportError:
        return
    mod = types.ModuleType("antenv.axon_hooks")
    mod._hook = _ntff_profile_via_ctypes("/opt/axon/libaxon_pjrt.so")

    def set_axon_ntff_profile_hook(h):
        mod._hook = h

    def get_axon_ntff_profile_hook():
        return mod._hook

    mod.set_axon_ntff_profile_hook = set_axon_ntff_profile_hook
    mod.get_axon_ntff_profile_hook = get_axon_ntff_profile_hook
    sys.modules["antenv.axon_hooks"] = mod
    import antenv
    antenv.axon_hooks = mod


def _run(inputs, trace=False):
    if trace:
        _install_profile_hook()
    in_maps, nkvs, newpos = _prep_inputs(inputs)
    nc = _get_program(nkvs, newpos)
    res = run_bass_kernel_spmd(nc, in_maps, core_ids=list(range(NC)), trace=trace)
    acc = np.zeros((BSZ * SEQ, DIM), np.float32)
    for c in range(NC):
        acc += res.results[c]["out"].astype(np.float32)
    return acc.reshape(BSZ, SEQ, DIM), res


def kernel(**inputs):
    out, _ = _run(inputs, trace=False)
    return out
